# revision 4
# baseline (speedup 1.0000x reference)
import numpy as np

NU, NL, NT, NA, D, LAYERS, B = 50000, 25000, 2048, 4096, 64, 3, 4096
NC = 8
P = 128

# table name -> full size
TABS = {"u_l": NU, "u_t": NU, "u_a": NA and NU, "l": NL, "t": NT, "a": NA}


def _shard(n):
    s = n // NC
    pad = ((s + P - 1) // P) * P
    return s, pad


DENSE = {"HG_t": "t", "VtoE_tu": "t", "HG_a": "a", "VtoE_ua": None, "VtoE_au": "a"}
DENSE = {"HG_t", "VtoE_tu", "HG_a", "VtoE_au"}  # matrices run as dense matmuls (small src tables)
# spmm: (matrix_prefix, out_table, src_table, primary?)
SPMMS = [
    ("HG_ul", "u_l", "u_l", True), ("VtoE_lu", "u_l", "l", False),
    ("HG_l", "l", "l", True), ("VtoE_ul", "l", "u_l", False),
    ("HG_ut", "u_t", "u_t", True), ("VtoE_tu", "u_t", "t", False),
    ("HG_t", "t", "t", True), ("VtoE_ut", "t", "u_t", False),
    ("HG_ua", "u_a", "u_a", True), ("VtoE_au", "u_a", "a", False),
    ("HG_a", "a", "a", True), ("VtoE_ua", "a", "u_a", False),
]


def _build_ell(rows, cols, vals, n_out, src_map, core):
    """Per-core ELL for one spmm. Returns dict with per-core data and structure."""
    s, pad = _shard(n_out)
    lo = core * s
    m = (rows >= lo) & (rows < lo + s)
    r = (rows[m] - lo).astype(np.int64)
    c = cols[m].astype(np.int64)
    v = vals[m].astype(np.float32)
    deg = np.bincount(r, minlength=s)
    order = np.argsort(-deg, kind="stable").astype(np.int64)  # local rows, sorted desc degree
    rank = np.empty(s, np.int64)
    rank[order] = np.arange(s)
    rr = rank[r]
    srt = np.argsort(rr, kind="stable")
    rr_s, c_s, v_s = rr[srt], c[srt], v[srt]
    starts = np.searchsorted(rr_s, np.arange(s))
    within = np.arange(len(rr_s)) - starts[rr_s]
    G = pad // P
    deg_sorted = np.zeros(pad, np.int64)
    deg_sorted[:s] = deg[order]
    w_g = np.maximum(deg_sorted[0::P], 1)  # per-group width (this core)
    return {
        "order": order, "rank": rank, "s": s, "pad": pad, "G": G, "w_g": w_g,
        "rr": rr_s, "col": c_s, "val": v_s, "within": within, "src_map": src_map,
    }


def _fill_streams(e, w_common, posmap_src0, posmap_src):
    """Build flat [sum(128*W_g)] idx(1), idx(23), val streams, p-major per group."""
    G = e["G"]
    base = np.zeros(G + 1, np.int64)
    base[1:] = np.cumsum(P * w_common)
    tot = base[-1]
    idx1 = np.zeros(tot, np.int32)
    idx23 = np.zeros(tot, np.int32)
    val = np.zeros(tot, np.float32)
    g = e["rr"] // P
    p = e["rr"] % P
    pos = base[g] + p * w_common[g] + e["within"]
    idx1[pos] = posmap_src0[e["col"]] if posmap_src0 is not None else e["col"]
    idx23[pos] = posmap_src[e["col"]]
    val[pos] = e["val"]
    return idx1, idx23, val, base, tot


def kernel_build(**inputs):
    import concourse.bacc as bacc
    import concourse.bass as bass
    import concourse.mybir as mybir
    import concourse.tile as tile
    from concourse.bass_utils import run_bass_kernel_spmd
    from concourse.masks import make_identity

    inp = {k: np.asarray(v) for k, v in inputs.items()}
    f32 = mybir.dt.float32
    i32 = mybir.dt.int32

    # ---------- host prep ----------
    # layer-0 projections
    ue = inp["user_emb"].astype(np.float32)
    t0 = {
        "u_l": ue @ inp["disen_l_w"].T.astype(np.float32) + inp["disen_l_b"],
        "u_t": ue @ inp["disen_t_w"].T.astype(np.float32) + inp["disen_t_b"],
        "u_a": ue @ inp["disen_a_w"].T.astype(np.float32) + inp["disen_a_b"],
        "l": inp["loc_emb"].astype(np.float32),
        "t": inp["time_emb"].astype(np.float32),
        "a": inp["act_emb"].astype(np.float32),
    }

    # per-core ELL structures for gather-path spmms
    ells = {}  # (mat) -> list per core
    for mat, out_tab, src_tab, is_pri in SPMMS:
        if mat in DENSE:
            continue
        n_out = TABS[out_tab]
        rows = np.asarray(inp[mat + "_row"]).astype(np.int64)
        cols = np.asarray(inp[mat + "_col"]).astype(np.int64)
        vals = np.asarray(inp[mat + "_val"]).astype(np.float32)
        ells[mat] = [_build_ell(rows, cols, vals, n_out, src_tab, c) for c in range(NC)]

    # common per-group widths across cores
    w_com = {mat: np.max(np.stack([e["w_g"] for e in ells[mat]]), axis=0)
             for mat, _, _, _ in SPMMS if mat not in DENSE}

    # global layouts (lambda / posmap) per table, defined by PRIMARY spmm's order
    posmap = {}  # table -> orig global id -> stored pos in [8*pad]
    lam = {}     # table -> stored pos -> orig global id
    for mat, out_tab, _, is_pri in SPMMS:
        if not is_pri:
            continue
        if mat in DENSE:
            n = TABS[out_tab]
            posmap[out_tab] = np.arange(n, dtype=np.int64)
            lam[out_tab] = np.arange(n, dtype=np.int64)
            continue
        s, pad = _shard(TABS[out_tab])
        pm = np.zeros(NC * pad, np.int64)
        lm = np.zeros(NC * pad, np.int64)
        pmap = np.zeros(TABS[out_tab], np.int64)
        for c in range(NC):
            e = ells[mat][c]
            orig = c * s + e["order"]  # stored j -> orig id (first s entries)
            lm[c * pad: c * pad + s] = orig
            lm[c * pad + s:(c + 1) * pad] = c * s  # dummy
            pmap[orig] = c * pad + np.arange(s)
        posmap[out_tab] = pmap
        lam[out_tab] = lm

    posmap0 = {x: None for x in TABS}

    # streams per spmm per core
    streams = {}
    for mat, out_tab, src_tab, is_pri in SPMMS:
        if mat in DENSE:
            continue
        pm0 = posmap0[src_tab]
        pms = posmap[src_tab].astype(np.int32)
        per_core = [
            _fill_streams(ells[mat][c], w_com[mat],
                          pm0.astype(np.int32) if pm0 is not None else None, pms)
            for c in range(NC)
        ]
        streams[mat] = per_core

    # merge maps for secondaries: map21[i] = sec stored pos of row at pri pos i (local)
    merge_map = {}
    for mat, out_tab, _, is_pri in SPMMS:
        if is_pri or mat in DENSE:
            continue
        s, pad = _shard(TABS[out_tab])
        pri_mat = next(m for m, ot, _, p in SPMMS if p and ot == out_tab)
        per_core = []
        for c in range(NC):
            sec_rank = ells[mat][c]["rank"]
            if pri_mat in DENSE:
                pri_order = np.arange(s, dtype=np.int64)
            else:
                pri_order = ells[pri_mat][c]["order"]
            m21 = np.arange(pad, dtype=np.int32)
            m21[:s] = sec_rank[pri_order].astype(np.int32)
            per_core.append(m21)
        merge_map[mat] = per_core

    # dense A^T slabs per core: AT[c_src, out_row_stored] bf16, duplicates summed
    import ml_dtypes
    at_slabs = {}
    for mat, out_tab, src_tab, is_pri in SPMMS:
        if mat not in DENSE:
            continue
        n_out = TABS[out_tab]
        C = TABS[src_tab]
        s_sh, pad = _shard(n_out)
        rows = np.asarray(inp[mat + "_row"]).astype(np.int64)
        cols = np.asarray(inp[mat + "_col"]).astype(np.int64)
        vals = np.asarray(inp[mat + "_val"]).astype(np.float32)
        per_core = []
        for c in range(NC):
            m = (rows >= c * s_sh) & (rows < (c + 1) * s_sh)
            r = rows[m] - c * s_sh
            if is_pri:
                tgt = r  # canonical order
            else:
                pri_mat = next(mm for mm, ot, _, p in SPMMS if p and ot == out_tab)
                tgt = ells[pri_mat][c]["rank"][r]  # stored order of primary
            AT = np.zeros((C, pad), np.float32)
            np.add.at(AT, (cols[m], tgt), vals[m])
            KC, G = C // P, pad // P
            ATB = AT.reshape(KC, P, G, P).transpose(2, 0, 1, 3).reshape(G * KC * P, P)
            per_core.append(np.ascontiguousarray(ATB.astype(ml_dtypes.bfloat16)))
        at_slabs[mat] = per_core

    # final-stage index arrays per core: [4, 512] per gather target
    users = np.asarray(inp["users"]).astype(np.int64)
    locs = np.asarray(inp["locations"]).astype(np.int64)
    tims = np.asarray(inp["times"]).astype(np.int64)
    BS = B // NC

    def fin_idx(tabname, ids):
        out = np.zeros((NC, 4, BS), np.int32)
        for c in range(NC):
            sl = ids[c * BS:(c + 1) * BS]
            base0 = posmap0[tabname]
            out[c, 0] = (base0[sl] if base0 is not None else sl).astype(np.int32)
            for k in (1, 2, 3):
                out[c, k] = posmap[tabname][sl].astype(np.int32)
        return out

    fin = {
        "u_l": fin_idx("u_l", users), "u_t": fin_idx("u_t", users),
        "u_a": fin_idx("u_a", users), "l": fin_idx("l", locs), "t": fin_idx("t", tims),
    }

    # ---------- build bass kernel ----------
    nc = bacc.Bacc("TRN2", target_bir_lowering=False, debug=False, num_devices=NC)

    dr_t0 = {x: nc.dram_tensor(f"t0_{x}", list(t0[x].shape), f32, kind="ExternalInput") for x in TABS}
    dr_gidx1, dr_gidx23, dr_gval, dr_m21 = {}, {}, {}, {}
    dr_at = {}
    for mat, out_tab, src_tab, is_pri in SPMMS:
        if mat in DENSE:
            dr_at[mat] = nc.dram_tensor(f"at_{mat}", list(at_slabs[mat][0].shape),
                                        mybir.dt.bfloat16, kind="ExternalInput")
            continue
        tot = streams[mat][0][4]
        dr_gidx1[mat] = nc.dram_tensor(f"g1_{mat}", [tot], i32, kind="ExternalInput")
        dr_gidx23[mat] = nc.dram_tensor(f"g23_{mat}", [tot], i32, kind="ExternalInput")
        dr_gval[mat] = nc.dram_tensor(f"gv_{mat}", [tot], f32, kind="ExternalInput")
        if not is_pri:
            _, pad = _shard(TABS[out_tab])
            dr_m21[mat] = nc.dram_tensor(f"m21_{mat}", [pad], i32, kind="ExternalInput")
    dr_fin = {x: nc.dram_tensor(f"fin_{x}", [4 * BS], i32, kind="ExternalInput") for x in fin}
    scores_out = nc.dram_tensor("scores", [BS, NA], f32, kind="ExternalOutput")

    # internal DRAM
    stg, stg2, tabk = {}, {}, {}
    for x in TABS:
        s, pad = _shard(TABS[x])
        stg[x] = nc.dram_tensor(f"stg_{x}", [pad, D], f32)
        stg2[x] = nc.dram_tensor(f"stg2_{x}", [pad, D], f32)
        for k in (1, 2, 3):
            tabk[(x, k)] = nc.dram_tensor(f"tab_{x}_{k}", [NC * pad, D], f32, addr_space="Shared")

    def src_ap(x, k):
        return dr_t0[x][:] if k == 1 else tabk[(x, k - 1)][:]

    with tile.TileContext(nc) as tc:
        with (
            tc.tile_pool(name="gat", bufs=3) as pool_gat,
            tc.tile_pool(name="prod", bufs=2) as pool_prod,
            tc.tile_pool(name="small", bufs=6) as pool_sm,
            tc.tile_pool(name="fin", bufs=2) as pool_fin,
            tc.tile_pool(name="psum", bufs=2, space="PSUM") as pool_ps,
            tc.tile_pool(name="const", bufs=1) as pool_c,
            tc.tile_pool(name="dns", bufs=3) as pool_dns,
            tc.tile_pool(name="psd", bufs=2, space="PSUM") as pool_psd,
        ):
            ident = pool_c.tile([P, P], f32)
            make_identity(nc, ident[:])

            def emit_spmm(mat, out_tab, src_tab, is_pri, k):
                wg = w_com[mat]
                base = streams[mat][0][3]
                gsrc = src_ap(src_tab, k)
                gidx = dr_gidx1[mat] if k == 1 else dr_gidx23[mat]
                dst = stg[out_tab] if is_pri else stg2[out_tab]
                for g in range(len(wg)):
                    W = int(wg[g])
                    o = int(base[g])
                    idx_t = pool_sm.tile([P, W], i32, tag="idx")
                    val_t = pool_sm.tile([P, W], f32, tag="val")
                    nc.sync.dma_start(idx_t[:], gidx[o:o + P * W].rearrange("(p w) -> p w", p=P))
                    nc.sync.dma_start(val_t[:], dr_gval[mat][o:o + P * W].rearrange("(p w) -> p w", p=P))
                    gat_t = pool_gat.tile([P, W, D], f32, tag="gat")
                    for w in range(W):
                        nc.gpsimd.indirect_dma_start(
                            out=gat_t[:, w, :], out_offset=None, in_=gsrc,
                            in_offset=bass.IndirectOffsetOnAxis(ap=idx_t[:, w:w + 1], axis=0),
                        )
                    prod_t = pool_prod.tile([P, W, D], f32, tag="prod")
                    nc.vector.tensor_tensor(
                        out=prod_t[:], in0=gat_t[:],
                        in1=val_t[:].unsqueeze(2).to_broadcast([P, W, D]),
                        op=mybir.AluOpType.mult,
                    )
                    acc_t = pool_sm.tile([P, D], f32, tag="acc")
                    nc.vector.tensor_reduce(
                        out=acc_t[:], in_=prod_t[:].transpose([0, 2, 1]),
                        axis=mybir.AxisListType.X, op=mybir.AluOpType.add,
                    )
                    nc.sync.dma_start(dst[g * P:(g + 1) * P, :], acc_t[:])

            def emit_merge(mat, out_tab):
                # stg[i] += stg2[m21[i]] ; contiguous read/write of stg, indirect gather of stg2
                _, pad = _shard(TABS[out_tab])
                for g in range(pad // P):
                    m_t = pool_sm.tile([P, 1], i32, tag="midx")
                    nc.sync.dma_start(m_t[:], dr_m21[mat][g * P:(g + 1) * P].rearrange("(p w) -> p w", p=P))
                    sec_t = pool_sm.tile([P, D], f32, tag="sec")
                    nc.gpsimd.indirect_dma_start(
                        out=sec_t[:], out_offset=None, in_=stg2[out_tab][:],
                        in_offset=bass.IndirectOffsetOnAxis(ap=m_t[:], axis=0),
                    )
                    pri_t = pool_sm.tile([P, D], f32, tag="pri")
                    nc.sync.dma_start(pri_t[:], stg[out_tab][g * P:(g + 1) * P, :])
                    nc.vector.tensor_tensor(out=pri_t[:], in0=pri_t[:], in1=sec_t[:], op=mybir.AluOpType.add)
                    nc.sync.dma_start(stg[out_tab][g * P:(g + 1) * P, :], pri_t[:])

            bf16 = mybir.dt.bfloat16

            def emit_dense(mat, out_tab, src_tab, is_pri, k):
                _, pad = _shard(TABS[out_tab])
                G = pad // P
                C = TABS[src_tab]
                KC = C // P
                tf = pool_dns.tile([P, KC, D], f32, tag="tf")
                nc.sync.dma_start(tf[:], src_ap(src_tab, k).rearrange("(a p) d -> p a d", p=P))
                tb = pool_dns.tile([P, KC, D], bf16, tag="tb")
                nc.vector.tensor_copy(out=tb[:], in_=tf[:])
                for g in range(G):
                    ps = pool_psd.tile([P, D], f32, tag="dmm")
                    for kc in range(KC):
                        at = pool_dns.tile([P, P], bf16, tag="at")
                        blk = (g * KC + kc) * P
                        nc.sync.dma_start(at[:], dr_at[mat][blk:blk + P, :])
                        nc.tensor.matmul(ps[:], lhsT=at[:], rhs=tb[:, kc, :],
                                         start=(kc == 0), stop=(kc == KC - 1))
                    o = pool_sm.tile([P, D], f32, tag="acc")
                    if is_pri:
                        nc.vector.tensor_copy(out=o[:], in_=ps[:])
                    else:
                        cur = pool_sm.tile([P, D], f32, tag="pri")
                        nc.sync.dma_start(cur[:], stg[out_tab][g * P:(g + 1) * P, :])
                        nc.vector.tensor_tensor(out=o[:], in0=cur[:], in1=ps[:], op=mybir.AluOpType.add)
                    nc.sync.dma_start(stg[out_tab][g * P:(g + 1) * P, :], o[:])

            for k in (1, 2, 3):
                for x in TABS:
                    pri_mat = next(m for m, ot, _, p in SPMMS if p and ot == x)
                    sec_mat = next(m for m, ot, _, p in SPMMS if (not p) and ot == x)
                    pri_src = next(st for m, _, st, p in SPMMS if m == pri_mat)
                    sec_src = next(st for m, _, st, p in SPMMS if m == sec_mat)
                    if pri_mat in DENSE:
                        emit_dense(pri_mat, x, pri_src, True, k)
                    else:
                        emit_spmm(pri_mat, x, pri_src, True, k)
                    if sec_mat in DENSE:
                        emit_dense(sec_mat, x, sec_src, False, k)
                    else:
                        emit_spmm(sec_mat, x, sec_src, False, k)
                        emit_merge(sec_mat, x)
                    nc.gpsimd.collective_compute(
                        "AllGather", mybir.AluOpType.bypass,
                        replica_groups=[list(range(NC))],
                        ins=[stg[x][:]], outs=[tabk[(x, k)][:]],
                    )

            # ---------- final stage ----------
            NBT = BS // P  # batch tiles per core
            # mean-sum gathers: for each needed vector, gather 4 tables and reduce
            def gather_sum(tabname, bt):
                gt = pool_fin.tile([P, 4, D], f32, tag="gsum")
                for k in range(4):
                    it = pool_sm.tile([P, 1], i32, tag="fidx")
                    o = k * BS + bt * P
                    nc.sync.dma_start(it[:], dr_fin[tabname][o:o + P].rearrange("(p w) -> p w", p=P))
                    nc.gpsimd.indirect_dma_start(
                        out=gt[:, k, :], out_offset=None,
                        in_=(dr_t0[tabname][:] if k == 0 else tabk[(tabname, k)][:]),
                        in_offset=bass.IndirectOffsetOnAxis(ap=it[:], axis=0),
                    )
                st = pool_fin.tile([P, D], f32, tag="gsumr")
                nc.vector.tensor_reduce(
                    out=st[:], in_=gt[:].transpose([0, 2, 1]),
                    axis=mybir.AxisListType.X, op=mybir.AluOpType.add,
                )
                return st

            # a-table sum: [4096] rows as [128, 32, 64]
            NAG = NC * _shard(NA)[1] // P  # 32
            asum = pool_c.tile([P, NAG, D], f32)
            for k in range(4):
                at = pool_fin.tile([P, NAG, D], f32, tag="atab")
                src = dr_t0["a"][:] if k == 0 else tabk[("a", k)][:]
                nc.sync.dma_start(at[:], src.rearrange("(a p) d -> p a d", p=P))
                if k == 0:
                    nc.vector.tensor_copy(out=asum[:], in_=at[:])
                else:
                    nc.vector.tensor_tensor(out=asum[:], in0=asum[:], in1=at[:], op=mybir.AluOpType.add)
            # transpose asum -> A_T [64, 4096]
            a_T = pool_c.tile([64, NAG * P], f32)
            for a in range(NAG):
                pt = pool_ps.tile([64, P], f32, tag="tp")
                nc.tensor.transpose(pt[:], asum[:, a, :], ident[:])
                nc.vector.tensor_copy(out=a_T[:, a * P:(a + 1) * P], in_=pt[:])

            for bt in range(NBT):
                uel = gather_sum("u_l", bt)
                le = gather_sum("l", bt)
                uet = gather_sum("u_t", bt)
                te = gather_sum("t", bt)
                uea = gather_sum("u_a", bt)
                # dot terms
                pr1 = pool_fin.tile([P, D], f32, tag="pr1")
                nc.vector.tensor_tensor(out=pr1[:], in0=uel[:], in1=le[:], op=mybir.AluOpType.mult)
                dot = pool_fin.tile([P, 1], f32, tag="dot")
                nc.vector.tensor_reduce(out=dot[:], in_=pr1[:], axis=mybir.AxisListType.X, op=mybir.AluOpType.add)
                pr2 = pool_fin.tile([P, D], f32, tag="pr2")
                nc.vector.tensor_tensor(out=pr2[:], in0=uet[:], in1=te[:], op=mybir.AluOpType.mult)
                dot2 = pool_fin.tile([P, 1], f32, tag="dot2")
                nc.vector.tensor_reduce(out=dot2[:], in_=pr2[:], axis=mybir.AxisListType.X, op=mybir.AluOpType.add)
                nc.vector.tensor_tensor(out=dot[:], in0=dot[:], in1=dot2[:], op=mybir.AluOpType.add)
                dotb = pool_fin.tile([P, 1], f32, tag="dotb")
                nc.vector.tensor_scalar_mul(dotb[:], dot[:], 1.0 / 16.0)
                # transpose uea
                ueaT_ps = pool_ps.tile([64, P], f32, tag="tp")
                nc.tensor.transpose(ueaT_ps[:], uea[:], ident[:])
                ueaT = pool_fin.tile([64, P], f32, tag="ueaT")
                nc.vector.tensor_copy(out=ueaT[:], in_=ueaT_ps[:])
                for jc in range(NA // 512):
                    mm = pool_ps.tile([P, 512], f32, tag="mm")
                    nc.tensor.matmul(mm[:], lhsT=ueaT[:], rhs=a_T[:, jc * 512:(jc + 1) * 512], start=True, stop=True)
                    res = pool_fin.tile([P, 512], f32, tag="res")
                    nc.scalar.activation(res[:], mm[:], mybir.ActivationFunctionType.Sigmoid,
                                         bias=dotb[:], scale=1.0 / 16.0)
                    nc.sync.dma_start(scores_out[bt * P:(bt + 1) * P, jc * 512:(jc + 1) * 512], res[:])

    nc.compile()

    # ---------- per-core inputs ----------
    in_maps = []
    for c in range(NC):
        im = {f"t0_{x}": np.ascontiguousarray(t0[x]) for x in TABS}
        for mat, out_tab, src_tab, is_pri in SPMMS:
            if mat in DENSE:
                im[f"at_{mat}"] = np.ascontiguousarray(at_slabs[mat][c])
                continue
            i1, i23, vv, _, _ = streams[mat][c]
            im[f"g1_{mat}"] = i1
            im[f"g23_{mat}"] = i23
            im[f"gv_{mat}"] = vv
            if not is_pri:
                im[f"m21_{mat}"] = merge_map[mat][c]
        for x in fin:
            im[f"fin_{x}"] = fin[x][c].reshape(-1)
        in_maps.append(im)

    def assemble(results):
        out = np.zeros((B, NA), np.float32)
        la = lam["a"][:NA]
        for c in range(NC):
            out[c * BS:(c + 1) * BS, la] = results[c]["scores"]
        return out

    return nc, in_maps, assemble


def kernel(**inputs):
    from concourse.bass_utils import run_bass_kernel_spmd

    nc, in_maps, assemble = kernel_build(**inputs)
    res = run_bass_kernel_spmd(nc, in_maps, core_ids=list(range(NC)))
    globals()["LAST"] = res
    return assemble(res.results)



# revision 16
# speedup vs baseline: 1.1077x; 1.1077x over previous
import os
import numpy as np
import ml_dtypes

NU, NL, NT, NA, D, LAYERS, B = 50000, 25000, 2048, 4096, 64, 3, 4096
NC = 8
P = 128
BS = B // NC

NQ = int(os.environ.get("KNQ", "4"))          # SWDGE queues for gathers
XMAXI = int(os.environ.get("KXM", "1024"))    # max idxs per dma_gather (ring)
PAD_FRAC = 0.15

TABS = {"u_l": NU, "u_t": NU, "u_a": NU, "l": NL, "t": NT, "a": NA}
PACKED = {"u_l", "u_t", "u_a"}                # bf16 pair-packed gather sources
DENSE = {"VtoE_tu", "HG_t", "VtoE_au", "HG_a"}  # small-src mats on the PE
if os.environ.get("KDL", "0") == "1":
    DENSE |= {"VtoE_lu", "HG_l"}              # l-sourced mats on the PE too

SPMMS = [
    ("HG_ul", "u_l", "u_l", True), ("VtoE_lu", "u_l", "l", False),
    ("HG_l", "l", "l", True), ("VtoE_ul", "l", "u_l", False),
    ("HG_ut", "u_t", "u_t", True), ("VtoE_tu", "u_t", "t", False),
    ("HG_t", "t", "t", True), ("VtoE_ut", "t", "u_t", False),
    ("HG_ua", "u_a", "u_a", True), ("VtoE_au", "u_a", "a", False),
    ("HG_a", "a", "a", True), ("VtoE_ua", "a", "u_a", False),
]

TABLE_ORDER = ["u_l", "l", "u_t", "t", "u_a", "a"]
BF = ml_dtypes.bfloat16


def _shard(n):
    s = n // NC
    pad = ((s + P - 1) // P) * P
    return s, pad


def _ktab(x):
    _, pad = _shard(TABS[x])
    G = pad // P
    return ((G + 1) // 2) * P


def _build_ell(rows, cols, vals, n_out, core, identity_order=False):
    s, pad = _shard(n_out)
    lo = core * s
    m = (rows >= lo) & (rows < lo + s)
    r = (rows[m] - lo).astype(np.int64)
    c = cols[m].astype(np.int64)
    v = vals[m].astype(np.float32)
    deg = np.bincount(r, minlength=s)
    if identity_order:
        order = np.arange(s, dtype=np.int64)
    else:
        order = np.argsort(-deg, kind="stable").astype(np.int64)
    rank = np.empty(s, np.int64)
    rank[order] = np.arange(s)
    rr = rank[r]
    srt = np.argsort(rr, kind="stable")
    rr_s, c_s, v_s = rr[srt], c[srt], v[srt]
    starts = np.searchsorted(rr_s, np.arange(s))
    within = np.arange(len(rr_s)) - starts[rr_s]
    deg_sorted = np.zeros(pad, np.int64)
    deg_sorted[:s] = deg[order]
    w_g = np.maximum(deg_sorted.reshape(-1, P).max(axis=1), 1)
    return {"order": order, "rank": rank, "s": s, "pad": pad, "w_g": w_g,
            "rr": rr_s, "col": c_s, "val": v_s, "within": within}


def _plan_chunks(w_com, chmax):
    G = len(w_com)
    plan = []
    g = 0
    while g < G:
        W = int(w_com[g])
        if W > chmax:
            off = 0
            while off < W:
                plan.append((g, 1, min(chmax, W - off), off))
                off += chmax
            g += 1
            continue
        ng = 1
        useful = W
        while g + ng < G and (ng + 1) * W <= chmax:
            useful2 = useful + int(w_com[g + ng])
            if ((ng + 1) * W - useful2) > PAD_FRAC * useful2:
                break
            useful = useful2
            ng += 1
        plan.append((g, ng, W, 0))
        g += ng
    return plan


def _wrap16(flat_j):
    X = len(flat_j)
    w = flat_j.reshape(X // 16, 16).T
    return np.ascontiguousarray(np.tile(w, (8, 1)))


def _streams_for(e, plan, posmap_src, packed_src, pad_src):
    """idx (int16 wrapped, chunk-major cols) + bf16 val stream (chunk-major)."""
    ef = 2 if packed_src else 1
    K = ((pad_src // P + 1) // 2) * P if packed_src else 0
    tot_x = sum(ng * W for (_, ng, W, _) in plan)
    idx_flat = np.zeros(128 * tot_x, np.int64)
    val = np.zeros(128 * tot_x * ef, np.float32)
    g_of = e["rr"] // P
    p_of = e["rr"] % P
    G = e["pad"] // P
    xbase = np.zeros(len(plan), np.int64)
    vbase = np.zeros(len(plan), np.int64)
    b = 0
    for i, (_, ng, W, _) in enumerate(plan):
        xbase[i] = b
        vbase[i] = b * ef
        b += 128 * ng * W
    spans = [[] for _ in range(G)]
    for ci, (g0, ng, W, woff) in enumerate(plan):
        for gi in range(ng):
            spans[g0 + gi].append((ci, gi, woff, woff + W))
    posj = np.full(len(e["rr"]), -1, np.int64)
    posv = np.full(len(e["rr"]), -1, np.int64)
    for g in range(G):
        sel = np.where(g_of == g)[0]
        if len(sel) == 0:
            continue
        wn = e["within"][sel]
        for (ci, gi, wlo, whi) in spans[g]:
            m = (wn >= wlo) & (wn < whi)
            if not m.any():
                continue
            _, ng, W, _ = plan[ci]
            x = gi * W + (wn[m] - wlo)
            posj[sel[m]] = xbase[ci] + x * 128 + p_of[sel[m]]
            posv[sel[m]] = vbase[ci] + p_of[sel[m]] * (ng * W * ef) + x * ef
    assert (posj >= 0).all()
    s = posmap_src[e["col"]]
    if packed_src:
        c = s // pad_src
        r = s % pad_src
        h = (r >= K).astype(np.int64)
        q = r - h * K
        idx_flat[posj] = c * K + q
        val[posv + h] = e["val"]
    else:
        idx_flat[posj] = s
        val[posv] = e["val"]
    out_idx = np.zeros((128, 8 * tot_x), np.int16)
    col = 0
    b = 0
    for (_, ng, W, _) in plan:
        X = 128 * ng * W
        out_idx[:, col:col + X // 16] = _wrap16(idx_flat[b:b + X].astype(np.int16))
        col += X // 16
        b += X
    return out_idx, val.astype(BF)


def kernel_build(**inputs):
    import concourse.bacc as bacc
    import concourse.bass as bass
    import concourse.mybir as mybir
    import concourse.tile as tile
    from concourse.masks import make_identity

    inp = {k: np.asarray(v) for k, v in inputs.items()}
    f32 = mybir.dt.float32
    bf16 = mybir.dt.bfloat16
    i16 = mybir.dt.int16

    # ---------- host prep ----------
    ue = inp["user_emb"].astype(np.float32)
    t0 = {
        "u_l": ue @ inp["disen_l_w"].T.astype(np.float32) + inp["disen_l_b"],
        "u_t": ue @ inp["disen_t_w"].T.astype(np.float32) + inp["disen_t_b"],
        "u_a": ue @ inp["disen_a_w"].T.astype(np.float32) + inp["disen_a_b"],
        "l": inp["loc_emb"].astype(np.float32),
        "t": inp["time_emb"].astype(np.float32),
        "a": inp["act_emb"].astype(np.float32),
    }

    ells = {}
    for mat, out_tab, src_tab, is_pri in SPMMS:
        if mat in DENSE:
            continue
        rows = np.asarray(inp[mat + "_row"]).astype(np.int64)
        cols = np.asarray(inp[mat + "_col"]).astype(np.int64)
        vals = np.asarray(inp[mat + "_val"]).astype(np.float32)
        ells[mat] = [_build_ell(rows, cols, vals, TABS[out_tab], c)
                     for c in range(NC)]

    chmax = XMAXI // P
    w_com = {m: np.max(np.stack([e["w_g"] for e in ells[m]]), axis=0)
             for m in ells}
    plans = {m: _plan_chunks(w_com[m], chmax) for m in ells}

    # stored layout: gather-pri tables from degree sort; t/a identity
    posmap, lam = {}, {}
    for mat, out_tab, _, is_pri in SPMMS:
        if not is_pri:
            continue
        s, pad = _shard(TABS[out_tab])
        lm = np.zeros(NC * pad, np.int64)
        pmap = np.zeros(TABS[out_tab], np.int64)
        for c in range(NC):
            if mat in DENSE:
                order = np.arange(s, dtype=np.int64)
            else:
                order = ells[mat][c]["order"]
            orig = c * s + order
            lm[c * pad: c * pad + s] = orig
            lm[c * pad + s:(c + 1) * pad] = c * s
            pmap[orig] = c * pad + np.arange(s)
        posmap[out_tab] = pmap
        lam[out_tab] = lm

    # t0 tables in stored layout
    t0p = {}
    for x in TABS:
        s, pad = _shard(TABS[x])
        full = t0[x][lam[x]]
        if x in PACKED:
            K = _ktab(x)
            pk = np.zeros((NC * K, 2 * D), np.float32)
            for c in range(NC):
                sh = full[c * pad:(c + 1) * pad]
                pk[c * K:c * K + K, :D] = sh[:K]
                pk[c * K:c * K + (pad - K), D:] = sh[K:]
            t0p[x] = np.ascontiguousarray(pk.astype(BF))
        else:
            t0p[x] = np.ascontiguousarray(full.astype(np.float32))

    streams = {}
    for mat, out_tab, src_tab, is_pri in SPMMS:
        if mat in DENSE:
            continue
        _, pad_src = _shard(TABS[src_tab])
        streams[mat] = [_streams_for(ells[mat][c], plans[mat], posmap[src_tab],
                                     src_tab in PACKED, pad_src)
                        for c in range(NC)]

    # dense A^T slabs (bf16), blocked [G, KC, P, P]
    at_slabs = {}
    for mat, out_tab, src_tab, is_pri in SPMMS:
        if mat not in DENSE:
            continue
        n_out = TABS[out_tab]
        C = TABS[src_tab]
        s_sh, pad = _shard(n_out)
        KC, G = C // P, pad // P
        rows = np.asarray(inp[mat + "_row"]).astype(np.int64)
        cols = np.asarray(inp[mat + "_col"]).astype(np.int64)
        vals = np.asarray(inp[mat + "_val"]).astype(np.float32)
        pri_mat = next(mm for mm, ot, _, p in SPMMS if p and ot == out_tab)
        per_core = []
        for c in range(NC):
            m = (rows >= c * s_sh) & (rows < (c + 1) * s_sh)
            r = rows[m] - c * s_sh
            if is_pri:
                tgt = r
            else:
                tgt = ells[pri_mat][c]["rank"][r]
            AT = np.zeros((C, pad), np.float32)
            np.add.at(AT, (cols[m], tgt), vals[m])
            ATB = (AT.reshape(KC, P, G, P).transpose(2, 0, 1, 3)
                   .reshape(G * KC * P, P))
            per_core.append(np.ascontiguousarray(ATB.astype(BF)))
        at_slabs[mat] = per_core

    # merge maps for gather-secondaries
    merge_map = {}
    for mat, out_tab, _, is_pri in SPMMS:
        if is_pri or mat in DENSE:
            continue
        s, pad = _shard(TABS[out_tab])
        pri_mat = next(m for m, ot, _, p in SPMMS if p and ot == out_tab)
        per_core = []
        for c in range(NC):
            sec_rank = ells[mat][c]["rank"]
            if pri_mat in DENSE:
                pri_order = np.arange(s, dtype=np.int64)
            else:
                pri_order = ells[pri_mat][c]["order"]
            m21 = np.arange(pad, dtype=np.int64)
            m21[:s] = sec_rank[pri_order]
            per_core.append(_wrap16(m21.astype(np.int16)))
        merge_map[mat] = per_core

    # final-stage indices + half masks
    users = np.asarray(inp["users"]).astype(np.int64)
    locs = np.asarray(inp["locations"]).astype(np.int64)
    tims = np.asarray(inp["times"]).astype(np.int64)

    fin_idx, fin_msk = {}, {}
    for x, ids in (("u_l", users), ("u_t", users), ("u_a", users),
                   ("l", locs), ("t", tims)):
        s, pad = _shard(TABS[x])
        per_idx, per_msk = [], []
        for c in range(NC):
            sl = posmap[x][ids[c * BS:(c + 1) * BS]]
            if x in PACKED:
                K = _ktab(x)
                cc = sl // pad
                r = sl % pad
                h = (r >= K).astype(np.int64)
                q = r - h * K
                per_idx.append(_wrap16((cc * K + q).astype(np.int16)))
                CB = BS // P
                hm = h.reshape(CB, P).T
                mk = np.zeros((2, P, CB), np.float32)
                mk[0] = 1.0 - hm
                mk[1] = hm
                per_msk.append(mk.reshape(2, -1).astype(BF))
            else:
                per_idx.append(_wrap16(sl.astype(np.int16)))
                per_msk.append(None)
        fin_idx[x] = per_idx
        fin_msk[x] = per_msk

    # ---------- build bass kernel ----------
    nc = bacc.Bacc("TRN2", target_bir_lowering=False, debug=False,
                   num_devices=NC, num_swdge_queues=NQ)

    dr_t0, dr_idx, dr_val, dr_m21, dr_at = {}, {}, {}, {}, {}
    for x in TABS:
        dr_t0[x] = nc.dram_tensor(f"t0_{x}", list(t0p[x].shape),
                                  bf16 if x in PACKED else f32,
                                  kind="ExternalInput")
    for mat, out_tab, src_tab, is_pri in SPMMS:
        if mat in DENSE:
            dr_at[mat] = nc.dram_tensor(f"at_{mat}", list(at_slabs[mat][0].shape),
                                        bf16, kind="ExternalInput")
            continue
        ish = streams[mat][0][0].shape
        vsh = streams[mat][0][1].shape
        dr_idx[mat] = nc.dram_tensor(f"gi_{mat}", list(ish), i16, kind="ExternalInput")
        dr_val[mat] = nc.dram_tensor(f"gv_{mat}", [int(np.prod(vsh))], bf16,
                                     kind="ExternalInput")
        if not is_pri:
            _, pad = _shard(TABS[out_tab])
            dr_m21[mat] = nc.dram_tensor(f"m21_{mat}", [P, pad // 16], i16,
                                         kind="ExternalInput")
    dr_fin, dr_fmsk = {}, {}
    for x in fin_idx:
        dr_fin[x] = nc.dram_tensor(f"fin_{x}", [P, BS // 16], i16, kind="ExternalInput")
        if x in PACKED:
            dr_fmsk[x] = nc.dram_tensor(f"fmsk_{x}", [2 * BS], bf16, kind="ExternalInput")
    scores_out = nc.dram_tensor("scores", [BS, NA], f32, kind="ExternalOutput")

    stg, stg2, tabk = {}, {}, {}
    for x in TABS:
        s, pad = _shard(TABS[x])
        stg2[x] = nc.dram_tensor(f"stg2_{x}", [pad, D], f32)
        if x in PACKED:
            K = _ktab(x)
            stg[x] = nc.dram_tensor(f"stg_{x}", [K, 2 * D], bf16)
            for k in (1, 2, 3):
                tabk[(x, k)] = nc.dram_tensor(f"tab_{x}_{k}", [NC * K, 2 * D], bf16,
                                              addr_space="Shared")
        else:
            stg[x] = nc.dram_tensor(f"stg_{x}", [pad, D], f32)
            for k in (1, 2, 3):
                tabk[(x, k)] = nc.dram_tensor(f"tab_{x}_{k}", [NC * pad, D], f32,
                                              addr_space="Shared")

    def src_ap(x, k):
        return dr_t0[x][:] if k == 1 else tabk[(x, k - 1)][:]

    qctr = [0]

    def next_q():
        q = qctr[0] % NQ
        qctr[0] += 1
        return q

    with tile.TileContext(nc) as tc:
        with (
            tc.tile_pool(name="gat", bufs=4) as pool_gat,
            tc.tile_pool(name="prod", bufs=2) as pool_prod,
            tc.tile_pool(name="strm", bufs=2) as pool_strm,
            tc.tile_pool(name="accp", bufs=2) as pool_acc,
            tc.tile_pool(name="accq", bufs=1) as pool_acc2,
            tc.tile_pool(name="mrg", bufs=1) as pool_mrg,
            tc.tile_pool(name="fin", bufs=1) as pool_fin,
            tc.tile_pool(name="dns", bufs=2) as pool_dns,
            tc.tile_pool(name="psum", bufs=2, space="PSUM") as pool_ps,
            tc.tile_pool(name="psd", bufs=2, space="PSUM") as pool_psd,
            tc.tile_pool(name="const", bufs=1) as pool_c,
        ):
            ident = pool_c.tile([P, P], f32)
            make_identity(nc, ident[:])

            # zero the never-written hi-half tail of packed stg tables
            # (odd G: rows [GH*128, K) have no hi partner; DRAM is uninit)
            zt = pool_c.tile([P, D], bf16)
            nc.vector.memset(zt[:], 0)
            for x in sorted(PACKED):
                K = _ktab(x)
                _, pad_x = _shard(TABS[x])
                GH = pad_x // P - K // P
                if GH * P < K:
                    nc.sync.dma_start(
                        stg[x][GH * P:K, D:2 * D], zt[:K - GH * P, :])

            SB = 16  # chunks per stream load

            def emit_gather_reduce(mat, src_tab, k, acc):
                plan = plans[mat]
                packed = src_tab in PACKED
                ef = 2 if packed else 1
                E = 2 * D if packed else D
                sdt = bf16 if packed else f32
                ci = 0
                icol_all = 0
                vbase_all = 0
                while ci < len(plan):
                    batch = plan[ci:ci + SB]
                    bx = sum(ng * W for (_, ng, W, _) in batch)
                    idx_t = pool_strm.tile([P, bx * 8], i16, tag="idx")
                    nc.sync.dma_start(idx_t[:],
                                      dr_idx[mat][:, icol_all:icol_all + bx * 8])
                    val_t = pool_strm.tile([P, bx * ef], bf16, tag="val")
                    nc.sync.dma_start(val_t[:],
                                      dr_val[mat][vbase_all:vbase_all + P * bx * ef]
                                      .rearrange("(p y) -> p y", p=P))
                    icol_all += bx * 8
                    vbase_all += P * bx * ef
                    io = 0
                    vo = 0
                    for (g0, ng, W, woff) in batch:
                        X = ng * W
                        NI = P * X
                        gat_t = pool_gat.tile([P, X, E], sdt, tag="gat")
                        nc.gpsimd.dma_gather(
                            out_ap=gat_t[:], in_ap=src_ap(src_tab, k),
                            idxs_ap=idx_t[:, io:io + NI // 16],
                            num_idxs=NI, num_idxs_reg=NI, elem_size=E,
                            queue_num=next_q())
                        prod_t = pool_prod.tile([P, X * ef, D], f32, tag="prod")
                        gin = (gat_t[:].rearrange("p x (h d) -> p (x h) d", h=2)
                               if packed else gat_t[:])
                        nc.vector.tensor_tensor(
                            out=prod_t[:], in0=gin,
                            in1=val_t[:, vo:vo + X * ef].unsqueeze(2)
                            .to_broadcast([P, X * ef, D]),
                            op=mybir.AluOpType.mult)
                        red_in = (prod_t[:].rearrange("p (g we) d -> p g we d", g=ng)
                                  .transpose([0, 1, 3, 2]))
                        if woff == 0:
                            out_ap = (acc[:, g0 * D:(g0 + ng) * D]
                                      .rearrange("p (g d) -> p g d", g=ng))
                            nc.vector.tensor_reduce(out=out_ap, in_=red_in,
                                                    axis=mybir.AxisListType.X,
                                                    op=mybir.AluOpType.add)
                        else:
                            tmp = pool_strm.tile([P, 1, D], f32, tag="tmp")
                            nc.vector.tensor_reduce(out=tmp[:], in_=red_in,
                                                    axis=mybir.AxisListType.X,
                                                    op=mybir.AluOpType.add)
                            sl = acc[:, g0 * D:(g0 + 1) * D]
                            nc.vector.tensor_tensor(out=sl, in0=sl, in1=tmp[:, 0, :],
                                                    op=mybir.AluOpType.add)
                        io += NI // 16
                        vo += X * ef
                    ci += len(batch)

            def emit_dense(mat, out_tab, src_tab, is_pri, k, acc):
                _, pad = _shard(TABS[out_tab])
                G = pad // P
                C = TABS[src_tab]
                KC = C // P
                tb = pool_dns.tile([P, KC, D], bf16, tag="tb")
                nc.gpsimd.dma_start(tb[:], src_ap(src_tab, k)
                                    .rearrange("(a p) d -> p a d", p=P))
                for g in range(G):
                    at = pool_dns.tile([P, KC, P], bf16, tag="at")
                    blk = g * KC * P
                    nc.sync.dma_start(at[:], dr_at[mat][blk:blk + KC * P]
                                      .rearrange("(kc p) q -> p kc q", p=P))
                    ps = pool_psd.tile([P, D], f32, tag="dmm")
                    for kc in range(KC):
                        nc.tensor.matmul(ps[:], lhsT=at[:, kc, :],
                                         rhs=tb[:, kc, :],
                                         start=(kc == 0), stop=(kc == KC - 1))
                    sl = acc[:, g * D:(g + 1) * D]
                    if is_pri:
                        nc.vector.tensor_copy(out=sl, in_=ps[:])
                    else:
                        nc.vector.tensor_tensor(out=sl, in0=sl, in1=ps[:],
                                                op=mybir.AluOpType.add)

            KCB = 32   # contraction blocks for large-KC dense
            GB = 8     # out-groups batched per psum bank

            def emit_dense_big(mat, out_tab, src_tab, is_pri, k, acc):
                _, pad = _shard(TABS[out_tab])
                G = pad // P
                C = TABS[src_tab]
                KC = C // P
                for gb0 in range(0, G, GB):
                    gn = min(GB, G - gb0)
                    ps = pool_psd.tile([P, gn, D], f32, tag="dmmb")
                    for kb0 in range(0, KC, KCB):
                        kn = min(KCB, KC - kb0)
                        tbb = pool_dns.tile([P, kn, D], bf16, tag="tbb")
                        nc.gpsimd.dma_start(
                            tbb[:], src_ap(src_tab, k)
                            .rearrange("(a p) d -> p a d", p=P)[:, kb0:kb0 + kn, :])
                        for gi in range(gn):
                            at = pool_dns.tile([P, kn, P], bf16, tag="atb")
                            blk = (gb0 + gi) * KC * P + kb0 * P
                            nc.sync.dma_start(
                                at[:], dr_at[mat][blk:blk + kn * P]
                                .rearrange("(kc p) q -> p kc q", p=P))
                            for kc in range(kn):
                                nc.tensor.matmul(
                                    ps[:, gi, :], lhsT=at[:, kc, :],
                                    rhs=tbb[:, kc, :],
                                    start=(kb0 == 0 and kc == 0),
                                    stop=(kb0 + kn == KC and kc == kn - 1))
                    for gi in range(gn):
                        sl = acc[:, (gb0 + gi) * D:(gb0 + gi + 1) * D]
                        if is_pri:
                            nc.vector.tensor_copy(out=sl, in_=ps[:, gi, :])
                        else:
                            nc.vector.tensor_tensor(out=sl, in0=sl, in1=ps[:, gi, :],
                                                    op=mybir.AluOpType.add)

            def emit_table(x, k):
                s, pad = _shard(TABS[x])
                G = pad // P
                pri = next(m for m, ot, _, p in SPMMS if p and ot == x)
                sec = next(m for m, ot, _, p in SPMMS if (not p) and ot == x)
                pri_src = next(st for m, _, st, p in SPMMS if m == pri)
                sec_src = next(st for m, _, st, p in SPMMS if m == sec)

                def dense_fn(mat, src):
                    return (emit_dense_big if TABS[src] // P > KCB else emit_dense)

                acc = pool_acc.tile([P, G * D], f32, tag="acc")
                if pri in DENSE:
                    dense_fn(pri, pri_src)(pri, x, pri_src, True, k, acc)
                else:
                    emit_gather_reduce(pri, pri_src, k, acc)

                if sec in DENSE:
                    dense_fn(sec, sec_src)(sec, x, sec_src, False, k, acc)
                else:
                    acc2 = pool_acc2.tile([P, G * D], f32, tag="acc2")
                    emit_gather_reduce(sec, sec_src, k, acc2)
                    nc.sync.dma_start(
                        stg2[x][:].rearrange("(g p) d -> p g d", p=P),
                        acc2[:].rearrange("p (g d) -> p g d", g=G))
                    m_t = pool_mrg.tile([P, pad // 16], i16, tag="m21")
                    nc.sync.dma_start(m_t[:], dr_m21[sec][:])
                    sec_t = pool_mrg.tile([P, G, D], f32, tag="sec")
                    gstep = XMAXI // P
                    for g0 in range(0, G, gstep):
                        gn = min(gstep, G - g0)
                        nc.gpsimd.dma_gather(
                            out_ap=sec_t[:, g0:g0 + gn, :], in_ap=stg2[x][:],
                            idxs_ap=m_t[:, g0 * 8:(g0 + gn) * 8],
                            num_idxs=gn * P, num_idxs_reg=gn * P, elem_size=D,
                            queue_num=next_q())
                    nc.vector.tensor_tensor(
                        out=acc[:], in0=acc[:],
                        in1=sec_t[:].rearrange("p g d -> p (g d)"),
                        op=mybir.AluOpType.add)

                if x in PACKED:
                    K = _ktab(x)
                    GL = K // P
                    GH = G - GL
                    ob = pool_mrg.tile([P, G * D], bf16, tag="ob")
                    nc.vector.tensor_copy(out=ob[:], in_=acc[:])
                    sv = stg[x][:].rearrange("(g p) (h d) -> p g h d", p=P, h=2)
                    nc.sync.dma_start(
                        sv[:, :, 0, :],
                        ob[:, :GL * D].rearrange("p (g d) -> p g d", g=GL))
                    nc.sync.dma_start(
                        sv[:, :GH, 1, :],
                        ob[:, GL * D:].rearrange("p (g d) -> p g d", g=GH))
                else:
                    nc.sync.dma_start(
                        stg[x][:].rearrange("(g p) d -> p g d", p=P),
                        acc[:].rearrange("p (g d) -> p g d", g=G))
                nc.gpsimd.collective_compute(
                    "AllGather", mybir.AluOpType.bypass,
                    replica_groups=[list(range(NC))],
                    ins=[stg[x][:]], outs=[tabk[(x, k)][:]])

            for k in (1, 2, 3):
                for x in TABLE_ORDER:
                    emit_table(x, k)

            # ---------- final stage ----------
            CB = BS // P

            def gather_ksum(x):
                packed = x in PACKED
                E = 2 * D if packed else D
                sdt = bf16 if packed else f32
                it = pool_fin.tile([P, BS // 16], i16, tag="fidx")
                nc.sync.dma_start(it[:], dr_fin[x][:])
                gt = pool_fin.tile([P, 4, CB, E], sdt, tag="gk")
                for kk in range(4):
                    nc.gpsimd.dma_gather(
                        out_ap=gt[:, kk, :, :],
                        in_ap=(dr_t0[x][:] if kk == 0 else tabk[(x, kk)][:]),
                        idxs_ap=it[:], num_idxs=BS, num_idxs_reg=BS,
                        elem_size=E, queue_num=next_q())
                if packed:
                    u2 = pool_fin.tile([P, CB, 2 * D], f32, tag="u2")
                    nc.vector.tensor_reduce(
                        out=u2[:], in_=gt[:].transpose([0, 2, 3, 1]),
                        axis=mybir.AxisListType.X, op=mybir.AluOpType.add)
                    mk = pool_fin.tile([P, 2, CB], bf16, tag="mk")
                    nc.sync.dma_start(mk[:], dr_fmsk[x][:]
                                      .rearrange("(h p c) -> p h c", p=P, h=2))
                    st = pool_fin.tile([P, CB, D], f32, tag=f"ks_{x}")
                    u2v = u2[:].rearrange("p c (h d) -> p c h d", h=2)
                    nc.vector.tensor_tensor(
                        out=st[:], in0=u2v[:, :, 0, :],
                        in1=mk[:, 0, :].unsqueeze(2).to_broadcast([P, CB, D]),
                        op=mybir.AluOpType.mult)
                    hi = pool_fin.tile([P, CB, D], f32, tag="hi")
                    nc.vector.tensor_tensor(
                        out=hi[:], in0=u2v[:, :, 1, :],
                        in1=mk[:, 1, :].unsqueeze(2).to_broadcast([P, CB, D]),
                        op=mybir.AluOpType.mult)
                    nc.vector.tensor_tensor(out=st[:], in0=st[:], in1=hi[:],
                                            op=mybir.AluOpType.add)
                    return st
                st = pool_fin.tile([P, CB, D], f32, tag=f"ks_{x}")
                nc.vector.tensor_reduce(
                    out=st[:], in_=gt[:].transpose([0, 2, 3, 1]),
                    axis=mybir.AxisListType.X, op=mybir.AluOpType.add)
                return st

            uel = gather_ksum("u_l")
            le = gather_ksum("l")
            uet = gather_ksum("u_t")
            te = gather_ksum("t")
            uea = gather_ksum("u_a")

            pr = pool_fin.tile([P, CB, D], f32, tag="pr")
            nc.vector.tensor_tensor(out=pr[:], in0=uel[:], in1=le[:],
                                    op=mybir.AluOpType.mult)
            dot1 = pool_fin.tile([P, CB], f32, tag="dot1")
            nc.vector.tensor_reduce(out=dot1[:], in_=pr[:],
                                    axis=mybir.AxisListType.X, op=mybir.AluOpType.add)
            pr2 = pool_fin.tile([P, CB, D], f32, tag="pr2")
            nc.vector.tensor_tensor(out=pr2[:], in0=uet[:], in1=te[:],
                                    op=mybir.AluOpType.mult)
            dot2 = pool_fin.tile([P, CB], f32, tag="dot2")
            nc.vector.tensor_reduce(out=dot2[:], in_=pr2[:],
                                    axis=mybir.AxisListType.X, op=mybir.AluOpType.add)
            dot = pool_fin.tile([P, CB], f32, tag="dot")
            nc.vector.tensor_tensor(out=dot[:], in0=dot1[:], in1=dot2[:],
                                    op=mybir.AluOpType.add)
            dotb = pool_fin.tile([P, CB], f32, tag="dotb")
            nc.vector.tensor_scalar_mul(dotb[:], dot[:], 1.0 / 16.0)

            _, pad_a = _shard(NA)
            NAG = NC * pad_a // P
            asum = pool_c.tile([P, NAG, D], f32)
            at0 = pool_fin.tile([P, NAG, D], f32, tag="at0")
            nc.sync.dma_start(at0[:], dr_t0["a"][:].rearrange("(a p) d -> p a d", p=P))
            nc.vector.tensor_copy(out=asum[:], in_=at0[:])
            for kk in (1, 2, 3):
                at2 = pool_fin.tile([P, NAG, D], f32, tag="at0")
                nc.sync.dma_start(at2[:], tabk[("a", kk)][:]
                                  .rearrange("(a p) d -> p a d", p=P))
                nc.vector.tensor_tensor(out=asum[:], in0=asum[:], in1=at2[:],
                                        op=mybir.AluOpType.add)
            a_T = pool_c.tile([64, NAG * P], f32)
            for a in range(NAG):
                pt = pool_ps.tile([64, P], f32, tag="tp")
                nc.tensor.transpose(pt[:], asum[:, a, :], ident[:])
                nc.vector.tensor_copy(out=a_T[:, a * P:(a + 1) * P], in_=pt[:])

            for cb in range(CB):
                ueaT_ps = pool_ps.tile([64, P], f32, tag="tp")
                nc.tensor.transpose(ueaT_ps[:], uea[:, cb, :], ident[:])
                ueaT = pool_fin.tile([64, P], f32, tag="ueaT")
                nc.vector.tensor_copy(out=ueaT[:], in_=ueaT_ps[:])
                for jc in range(NA // 512):
                    mm = pool_ps.tile([P, 512], f32, tag="mm")
                    nc.tensor.matmul(mm[:], lhsT=ueaT[:],
                                     rhs=a_T[:, jc * 512:(jc + 1) * 512],
                                     start=True, stop=True)
                    res = pool_fin.tile([P, 512], f32, tag="res")
                    nc.scalar.activation(res[:], mm[:],
                                         mybir.ActivationFunctionType.Sigmoid,
                                         bias=dotb[:, cb:cb + 1], scale=1.0 / 16.0)
                    nc.sync.dma_start(
                        scores_out[cb * P:(cb + 1) * P, jc * 512:(jc + 1) * 512],
                        res[:])

    nc.compile()

    # ---------- per-core inputs ----------
    in_maps = []
    for c in range(NC):
        im = {f"t0_{x}": t0p[x] for x in TABS}
        for mat, out_tab, src_tab, is_pri in SPMMS:
            if mat in DENSE:
                im[f"at_{mat}"] = at_slabs[mat][c]
                continue
            idx, val = streams[mat][c]
            im[f"gi_{mat}"] = idx
            im[f"gv_{mat}"] = val.reshape(-1)
            if not is_pri:
                im[f"m21_{mat}"] = merge_map[mat][c]
        for x in fin_idx:
            im[f"fin_{x}"] = fin_idx[x][c]
            if x in PACKED:
                im[f"fmsk_{x}"] = fin_msk[x][c].reshape(-1)
        in_maps.append(im)

    la = lam["a"]

    def assemble(results):
        out = np.zeros((B, NA), np.float32)
        for c in range(NC):
            out[c * BS:(c + 1) * BS, la] = results[c]["scores"]
        return out

    return nc, in_maps, assemble


def kernel(**inputs):
    from concourse.bass_utils import run_bass_kernel_spmd

    nc, in_maps, assemble = kernel_build(**inputs)
    res = run_bass_kernel_spmd(nc, in_maps, core_ids=list(range(NC)))
    globals()["LAST"] = res
    return assemble(res.results)
